# revision 1
# baseline (speedup 1.0000x reference)
"""BitMLP (BitNet-style MLP) Trainium2 kernel, 8-way data-parallel over tokens.

reference semantics:
  h   = act_quant(rms_norm(x, gamma)) @ w1q.T   (w1q = per-tensor ternary quant)
  out = act_quant(gelu_exact(h)) @ w2q.T

Key facts exploited:
  * act_quant produces n/scale with n an integer in [-127, 127]  -> n is exact in bf16
  * weight quant produces t*inv_w with t ternary in {-1, 0, 1}   -> t is exact in bf16
  * therefore both matmuls are exact integer accumulations computed in bf16 at
    full TensorE rate; per-token/per-tensor scales are applied afterwards.

Sharding (8 cores on one chip):
  * tokens (4*2048 = 8192) split 1024/core; each core computes its tokens' output
  * weight quantization is cooperative: core c quantizes 1/8 of w1 and w2,
    the per-tensor mean(|w|) is combined with a scalar AllReduce, and the
    ternary bf16 weights are AllGathered (in a 128x128 block layout so the
    matmul weight loads are contiguous). w1's gather is split into 4 chunked
    AllGathers so mm1 can start on the first chunk.
  * on-chip layout is [feature(part), token(free)] throughout; per-token scales
    become broadcast rows (PE outer-product broadcast; GpSimd only runs
    collectives so broadcast never queues behind a collective wait).
  * Q2 (requantization of h) + MM2 run per token-half so Q2 of half 1
    overlaps MM2 of half 0 (w2 is streamed twice, which DMA affords).
"""

import os
import sys

for _p in ("/root/.axon_site/_ro/trn_rl_repo", "/opt/trn_rl_repo"):
    if os.path.isdir(_p) and _p not in sys.path:
        sys.path.append(_p)

from contextlib import ExitStack

import numpy as np

from concourse import bacc, bass, masks, mybir, tile
from concourse import bass_utils

F32 = mybir.dt.float32
BF16 = mybir.dt.bfloat16
AF = mybir.ActivationFunctionType
OP = mybir.AluOpType
AX = mybir.AxisListType

NCORES = 8
B, S, DIM, HID = 4, 2048, 2048, 8192
NTOK = B * S            # 8192
TOK = NTOK // NCORES    # 1024 tokens per core
KT = DIM // 128         # 16 k-tiles
HB = HID // 128         # 64 hid blocks
DB = DIM // 128         # 16 dim blocks
HBL = HID // NCORES // 128  # 8 hid blocks owned per core
NAG = 4                 # w1 AllGather split into NAG chunks
HBC = HBL // NAG        # hid blocks per chunk per core
MAGIC = 12582912.0      # 1.5 * 2**23: (v + MAGIC) - MAGIC == round-half-even(v)
EPS = 1e-6
W_NELEM = float(DIM * HID)

_cache = {}




def _build(n_cores=NCORES):
    nc = bacc.Bacc("TRN2", target_bir_lowering=False, debug=False, num_devices=n_cores)
    xT = nc.dram_tensor("xT", [DIM, TOK], F32, kind="ExternalInput")
    w1s = nc.dram_tensor("w1s", [DIM, HID // n_cores], F32, kind="ExternalInput")
    w2s = nc.dram_tensor("w2s", [HID // n_cores, DIM], F32, kind="ExternalInput")
    gpt = nc.dram_tensor("gpt", [128, KT], F32, kind="ExternalInput")
    outT = nc.dram_tensor("outT", [DIM, TOK], F32, kind="ExternalOutput")
    rg = [list(range(n_cores))]

    with tile.TileContext(nc) as tc, ExitStack() as ctx:
        misc = ctx.enter_context(tc.tile_pool(name="misc", bufs=1))
        hio = ctx.enter_context(tc.tile_pool(name="hio", bufs=3))
        ps_mm = ctx.enter_context(tc.tile_pool(name="ps_mm", bufs=2, space="PSUM"))
        ps_tr = ctx.enter_context(tc.tile_pool(name="ps_tr", bufs=2, space="PSUM"))
        ps_ss = ctx.enter_context(tc.tile_pool(name="ps_ss", bufs=1, space="PSUM"))
        dram = ctx.enter_context(tc.tile_pool(name="dram", bufs=1, space="DRAM"))

        ident = misc.tile([128, 128], F32)
        masks.make_identity(nc, ident[:])
        zero_col = misc.tile([128, 1], F32)
        nc.vector.memset(zero_col[:], 0.0)
        ones_bf = misc.tile([128, 1], BF16)
        nc.vector.memset(ones_bf[:], 1.0)
        ones_f = misc.tile([128, 1], F32)
        nc.vector.memset(ones_f[:], 1.0)
        ones_row = misc.tile([1, 128], F32)
        nc.vector.memset(ones_row[:], 1.0)
        gam = misc.tile([128, KT], F32)
        nc.sync.dma_start(gam[:], gpt[:])

        def bcast_row(dst, src_row, n, off=0):
            """dst[128, off:off+n] = broadcast of src_row[1, n] via PE outer product."""
            for o in range(0, n, 512):
                w = min(512, n - o)
                ps = ps_mm.tile([128, 512], F32, tag="mm0")
                nc.tensor.matmul(ps[:, 0:w], ones_row[:], src_row[:, o:o + w],
                                 start=True, stop=True)
                nc.scalar.activation(dst[:, off + o:off + o + w], ps[:, 0:w], AF.Copy, bias=0.0)

        # DRAM scratch
        ar_in = dram.tile([2, 1], F32)
        ar_out = dram.tile([2, 1], F32, addr_space="Shared")
        t1_store = [dram.tile([HBC, 128, KT * 128], BF16, name=f"t1s{i}") for i in range(NAG)]
        t1_g = [dram.tile([n_cores, HBC, 128, KT * 128], BF16, addr_space="Shared",
                          name=f"t1g{i}") for i in range(NAG)]
        t2_store = dram.tile([DB, 128, HBL * 128], BF16)
        t2_g = dram.tile([n_cores, DB, 128, HBL * 128], BF16, addr_space="Shared")
        h_dram = dram.tile([HB, 128, TOK], F32)
        r1_d = dram.tile([8, 128], F32)
        r2_d = dram.tile([8, 128], F32)

        # rows shared by late phases
        s2r = misc.tile([128, TOK], F32)
        i2r = misc.tile([128, TOK], F32)
        invw = misc.tile([1, 2], F32)

        # ================= Phase W: per-tensor |w| sums + AllReduce ========
        S12 = misc.tile([128, 2], F32)
        S1 = misc.tile([128, KT], F32)
        S2 = misc.tile([128, HBL], F32)
        with tc.tile_pool(name="wio", bufs=2) as wio:
            for kt in range(KT):
                wt = wio.tile([128, HID // n_cores], F32, tag="w1t")
                nc.sync.dma_start(wt[:], w1s[kt * 128:(kt + 1) * 128, :])
                nc.vector.tensor_reduce(S1[:, kt:kt + 1], wt[:], axis=AX.X, op=OP.add,
                                        apply_absolute_value=True)
            for ht in range(HBL):
                wt2 = wio.tile([128, DIM], F32, tag="w2t")
                nc.sync.dma_start(wt2[:], w2s[ht * 128:(ht + 1) * 128, :])
                nc.vector.tensor_reduce(S2[:, ht:ht + 1], wt2[:], axis=AX.X, op=OP.add,
                                        apply_absolute_value=True)
            nc.vector.tensor_reduce(S12[:, 0:1], S1[:], axis=AX.X, op=OP.add)
            nc.vector.tensor_reduce(S12[:, 1:2], S2[:], axis=AX.X, op=OP.add)
            tot_ps = ps_tr.tile([2, 1], F32, tag="tr")
            nc.tensor.matmul(tot_ps[:], S12[:], ones_f[:], start=True, stop=True)
            tot_sb = misc.tile([2, 1], F32)
            nc.vector.tensor_copy(tot_sb[:], tot_ps[:])
            nc.sync.dma_start(ar_in[:], tot_sb[:])
            nc.gpsimd.collective_compute(
                "AllReduce", OP.add, replica_groups=rg, ins=[ar_in[:]], outs=[ar_out[:]])
            tot2 = misc.tile([1, 2], F32)
            nc.sync.dma_start(tot2[:], ar_out[:].rearrange("a b -> b a"))
            # inv_w = max(mean|w|, 1e-5); scale_w = 1/inv_w
            nc.vector.tensor_scalar(invw[:], tot2[:], 1.0 / W_NELEM, 1e-5, op0=OP.mult, op1=OP.max)
            sw = misc.tile([1, 2], F32)
            nc.vector.reciprocal(sw[:], invw[:])
            swb = misc.tile([128, 2], F32)
            ps_b = ps_tr.tile([128, 2], F32, tag="tr")
            nc.tensor.matmul(ps_b[:], ones_row[:], sw[:], start=True, stop=True)
            nc.scalar.activation(swb[:], ps_b[:], AF.Copy, bias=0.0)

            # ============= Phase WQ: ternary-quantize shards + AllGather ====
            # w*sw -> clip[-1,1] -> round (clip-then-round == round-then-clip here)
            # chunk-outer so each chunked AllGather fires as soon as its
            # columns are quantized
            CW = HBC * 128
            for ci in range(NAG):
                for kt in range(KT):
                    wt = wio.tile([128, CW], F32, tag="w1t")
                    nc.sync.dma_start(wt[:], w1s[kt * 128:(kt + 1) * 128,
                                                 ci * CW:(ci + 1) * CW])
                    nc.vector.tensor_scalar(wt[:], wt[:], swb[:, 0:1], -1.0, op0=OP.mult, op1=OP.max)
                    nc.vector.tensor_scalar(wt[:], wt[:], 1.0, MAGIC, op0=OP.min, op1=OP.add)
                    q = wio.tile([128, CW], BF16, tag="wqq")
                    nc.vector.tensor_scalar(q[:], wt[:], MAGIC, None, op0=OP.subtract)
                    nc.gpsimd.dma_start(
                        t1_store[ci][:, :, kt * 128:(kt + 1) * 128].rearrange(
                            "b k j -> k b j"),
                        q[:].rearrange("k (b j) -> k b j", b=HBC))
                nc.gpsimd.collective_compute(
                    "AllGather", OP.bypass, replica_groups=rg,
                    ins=[t1_store[ci][:]], outs=[t1_g[ci][:]])
            for ht in range(HBL):
                wt2 = wio.tile([128, DIM], F32, tag="w2t")
                nc.sync.dma_start(wt2[:], w2s[ht * 128:(ht + 1) * 128, :])
                nc.vector.tensor_scalar(wt2[:], wt2[:], swb[:, 1:2], -1.0, op0=OP.mult, op1=OP.max)
                nc.vector.tensor_scalar(wt2[:], wt2[:], 1.0, MAGIC, op0=OP.min, op1=OP.add)
                q2 = wio.tile([128, DIM], BF16, tag="wqq2")
                nc.vector.tensor_scalar(q2[:], wt2[:], MAGIC, None, op0=OP.subtract)
                nc.gpsimd.dma_start(
                    t2_store[:, :, ht * 128:(ht + 1) * 128].rearrange("d k j -> k d j"),
                    q2[:].rearrange("k (d j) -> k d j", d=DB))
            nc.gpsimd.collective_compute(
                "AllGather", OP.bypass, replica_groups=rg, ins=[t2_store[:]], outs=[t2_g[:]])

        # ================= Phase X + MM1 ===================================
        with ExitStack() as xctx:
            xio = xctx.enter_context(tc.tile_pool(name="xio", bufs=3))
            scr = xctx.enter_context(tc.tile_pool(name="scr", bufs=3))
            xq = xctx.enter_context(tc.tile_pool(name="xq", bufs=1))
            w1st = xctx.enter_context(tc.tile_pool(name="w1st", bufs=3))
            rows = xctx.enter_context(tc.tile_pool(name="rows", bufs=1))
            xmisc = xctx.enter_context(tc.tile_pool(name="xmisc", bufs=1))

            acc = xmisc.tile([128, TOK], F32)
            nc.vector.memset(acc[:], 0.0)
            ss_ps0 = ps_ss.tile([1, 512], F32, tag="ss0")
            ss_ps1 = ps_ss.tile([1, 512], F32, tag="ss1")
            for kt in range(KT):
                xt = xio.tile([128, TOK], F32, tag="xt")
                nc.sync.dma_start(xt[:], xT[kt * 128:(kt + 1) * 128, :])
                x2 = scr.tile([128, TOK], BF16, tag="x2")
                nc.scalar.activation(x2[:], xt[:], AF.Square, bias=zero_col[:])
                nc.tensor.matmul(ss_ps0[:], ones_bf[:], x2[:, 0:512],
                                 start=(kt == 0), stop=(kt == KT - 1))
                nc.tensor.matmul(ss_ps1[:], ones_bf[:], x2[:, 512:1024],
                                 start=(kt == 0), stop=(kt == KT - 1))
                xg = scr.tile([128, TOK], F32, tag="xg")
                nc.vector.tensor_scalar(xg[:], xt[:], gam[:, kt:kt + 1], None, op0=OP.mult)
                xga = scr.tile([128, TOK], F32, tag="xga")
                nc.scalar.activation(xga[:], xg[:], AF.Abs, bias=zero_col[:])
                nc.vector.tensor_tensor(acc[:], acc[:], xga[:], op=OP.max)

            # per-token rstd row (1/sqrt(mean(x^2)+eps))
            v_row = rows.tile([1, TOK], F32)
            nc.vector.tensor_scalar(v_row[:, 0:512], ss_ps0[:], 1.0 / DIM, EPS, op0=OP.mult, op1=OP.add)
            nc.vector.tensor_scalar(v_row[:, 512:1024], ss_ps1[:], 1.0 / DIM, EPS, op0=OP.mult, op1=OP.add)
            sq_row = rows.tile([1, TOK], F32)
            nc.scalar.activation(sq_row[:], v_row[:], AF.Sqrt, bias=zero_col[0:1, :])
            rscr = rows.tile([1, TOK], F32)
            rstd_row = rows.tile([1, TOK], F32)
            nc.vector.reciprocal_approx_accurate(rstd_row[:], sq_row[:], rscr[:])

            # per-token absmax of xn: (max_k |x*gamma|) * rstd
            m0 = xmisc.tile([128, 8], F32)
            for c in range(8):
                pt = ps_tr.tile([128, 128], F32, tag="tr")
                nc.tensor.transpose(pt[:], acc[:, c * 128:(c + 1) * 128], ident[:])
                nc.vector.tensor_reduce(m0[:, c:c + 1], pt[:], axis=AX.X, op=OP.max)
            nc.sync.dma_start(r1_d[:].rearrange("c p -> p c"), m0[:])
            m0row = rows.tile([1, TOK], F32)
            nc.sync.dma_start(m0row[:], r1_d[:].rearrange("c p -> (c p)")[None, :])
            nc.vector.tensor_tensor(m0row[:], m0row[:], rstd_row[:], op=OP.mult)
            nc.vector.tensor_scalar(m0row[:], m0row[:], 1e-5, None, op0=OP.max)
            sx_row0 = rows.tile([1, TOK], F32)
            nc.vector.reciprocal_approx_accurate(sx_row0[:], m0row[:], rscr[:])
            nc.vector.tensor_scalar(sx_row0[:], sx_row0[:], 127.0, None, op0=OP.mult)
            inv_sx = rows.tile([1, TOK], F32)
            nc.vector.reciprocal_approx_accurate(inv_sx[:], sx_row0[:], rscr[:])
            rsx_row0 = rows.tile([1, TOK], F32)
            nc.vector.tensor_tensor(rsx_row0[:], rstd_row[:], sx_row0[:], op=OP.mult)
            s1_row0 = inv_sx
            nc.vector.tensor_scalar(s1_row0[:], inv_sx[:], invw[:, 0:1], None, op0=OP.mult)
            rsx = xmisc.tile([128, TOK], F32)
            bcast_row(rsx, rsx_row0, TOK)
            s1r = xmisc.tile([128, TOK], F32)
            bcast_row(s1r, s1_row0, TOK)

            # quantize: n_xT = round((x*gamma) * rstd*sx)  (bf16 ints)
            nxT = xq.tile([128, KT * TOK], BF16)
            for kt in range(KT):
                xt = xio.tile([128, TOK], F32, tag="xt")
                nc.sync.dma_start(xt[:], xT[kt * 128:(kt + 1) * 128, :])
                t = scr.tile([128, TOK], F32, tag="xg")
                nc.vector.scalar_tensor_tensor(t[:], xt[:], gam[:, kt:kt + 1], rsx[:],
                                               op0=OP.mult, op1=OP.mult)
                nc.vector.tensor_scalar(nxT[:, kt * TOK:(kt + 1) * TOK], t[:], MAGIC, MAGIC,
                                        op0=OP.add, op1=OP.subtract)

            # ---- MM1: h = gelu((n_x @ t1) * s1), absmax on the fly ---------
            acc2 = xmisc.tile([128, TOK], F32)
            nc.vector.memset(acc2[:], 0.0)
            for ci in range(NAG):
                for r in range(n_cores):
                    for bi in range(HBC):
                        hb = r * HBL + ci * HBC + bi
                        wb = w1st.tile([128, KT * 128], BF16, tag="wb")
                        nc.sync.dma_start(wb[:], t1_g[ci][r, bi])
                        ps0 = ps_mm.tile([128, 512], F32, tag="mm0")
                        ps1 = ps_mm.tile([128, 512], F32, tag="mm1")
                        for kt in range(KT):
                            st, sp = (kt == 0), (kt == KT - 1)
                            nc.tensor.matmul(ps0[:], wb[:, kt * 128:(kt + 1) * 128],
                                             nxT[:, kt * TOK:kt * TOK + 512], start=st, stop=sp)
                            nc.tensor.matmul(ps1[:], wb[:, kt * 128:(kt + 1) * 128],
                                             nxT[:, kt * TOK + 512:kt * TOK + 1024], start=st, stop=sp)
                        g = scr.tile([128, TOK], F32, tag="g")
                        for th, ps in ((0, ps0), (1, ps1)):
                            sl = slice(th * 512, th * 512 + 512)
                            hs = scr.tile([128, 512], F32, tag="hs")
                            nc.vector.tensor_tensor(hs[:], ps[:], s1r[:, sl], op=OP.mult)
                            nc.scalar.activation(g[:, sl], hs[:], AF.Gelu, bias=zero_col[:])
                            ga = scr.tile([128, 512], F32, tag="ga")
                            nc.scalar.activation(ga[:], g[:, sl], AF.Abs, bias=zero_col[:])
                            nc.vector.tensor_tensor(acc2[:, sl], acc2[:, sl], ga[:], op=OP.max)
                        nc.scalar.dma_start(h_dram[hb], g[:])

            # scale2 rows
            m2 = xmisc.tile([128, 8], F32)
            for c in range(8):
                pt = ps_tr.tile([128, 128], F32, tag="tr")
                nc.tensor.transpose(pt[:], acc2[:, c * 128:(c + 1) * 128], ident[:])
                nc.vector.tensor_reduce(m2[:, c:c + 1], pt[:], axis=AX.X, op=OP.max)
            nc.sync.dma_start(r2_d[:].rearrange("c p -> p c"), m2[:])
            m2row = rows.tile([1, TOK], F32)
            nc.sync.dma_start(m2row[:], r2_d[:].rearrange("c p -> (c p)")[None, :])
            nc.vector.tensor_scalar(m2row[:], m2row[:], 1e-5, None, op0=OP.max)
            s2_row0 = rows.tile([1, TOK], F32)
            nc.vector.reciprocal_approx_accurate(s2_row0[:], m2row[:], rscr[:])
            nc.vector.tensor_scalar(s2_row0[:], s2_row0[:], 127.0, None, op0=OP.mult)
            i2_row0 = rows.tile([1, TOK], F32)
            nc.vector.reciprocal_approx_accurate(i2_row0[:], s2_row0[:], rscr[:])
            nc.vector.tensor_scalar(i2_row0[:], i2_row0[:], invw[:, 1:2], None, op0=OP.mult)
            bcast_row(s2r, s2_row0, TOK)
            bcast_row(i2r, i2_row0, TOK)

        # ================= Q2 + MM2 (per token-half) =======================
        with ExitStack() as hctx:
            hqp = hctx.enter_context(tc.tile_pool(name="hqp", bufs=1))
            w2st = hctx.enter_context(tc.tile_pool(name="w2st", bufs=2))
            hbk = hctx.enter_context(tc.tile_pool(name="hbk", bufs=3))
            HK = HB // 2  # 32 k-blocks per w2 stream buffer
            for th in range(2):
                to = th * 512
                hqh = hqp.tile([128, HB * 512], BF16, tag=f"hq{th}")
                for hb2 in range(HB // 2):
                    hb = hb2 * 2
                    hb_t = hbk.tile([128, TOK], F32, tag="hback")
                    nc.gpsimd.dma_start(
                        hb_t[:].rearrange("k (b j) -> k b j", b=2),
                        h_dram[hb:hb + 2, :, to:to + 512].rearrange("b k j -> k b j"))
                    t2 = hbk.tile([128, TOK], F32, tag="ht2")
                    st2 = s2r[:, to:to + 512]
                    nc.vector.tensor_tensor(t2[:, 0:512], hb_t[:, 0:512], st2, op=OP.mult)
                    nc.vector.tensor_tensor(t2[:, 512:1024], hb_t[:, 512:1024], st2, op=OP.mult)
                    nc.vector.tensor_scalar(hqh[:, hb * 512:(hb + 2) * 512], t2[:],
                                            MAGIC, MAGIC, op0=OP.add, op1=OP.subtract)
                for d in range(DB):
                    nr2 = n_cores // 2
                    wA = w2st.tile([128, HK * 128], BF16, tag="wA")
                    nc.sync.dma_start(
                        wA[:].rearrange("k (r f) -> k r f", r=nr2),
                        t2_g[0:nr2, d].rearrange("r k f -> k r f"))
                    wB = w2st.tile([128, HK * 128], BF16, tag="wB")
                    nc.sync.dma_start(
                        wB[:].rearrange("k (r f) -> k r f", r=nr2),
                        t2_g[nr2:n_cores, d].rearrange("r k f -> k r f"))
                    ps = ps_mm.tile([128, 512], F32, tag=f"mm{th}")
                    for kg in range(HB):
                        st, sp = (kg == 0), (kg == HB - 1)
                        w_ = wA if kg < HK else wB
                        ko = (kg % HK) * 128
                        nc.tensor.matmul(ps[:], w_[:, ko:ko + 128],
                                         hqh[:, kg * 512:(kg + 1) * 512], start=st, stop=sp)
                    ot = hio.tile([128, 512], F32, tag="ot")
                    nc.vector.tensor_tensor(ot[:], ps[:], i2r[:, to:to + 512], op=OP.mult)
                    nc.sync.dma_start(outT[d * 128:(d + 1) * 128, to:to + 512], ot[:])

    nc.compile()
    return nc


def _get_nc():
    if "nc" not in _cache:
        _cache["nc"] = _build()
    return _cache["nc"]


def _prep_inputs(x, w1, w2, gamma):
    x2d = np.ascontiguousarray(np.asarray(x, dtype=np.float32).reshape(NTOK, DIM))
    w1 = np.asarray(w1, dtype=np.float32)
    w2 = np.asarray(w2, dtype=np.float32)
    gamma = np.asarray(gamma, dtype=np.float32)
    w1T = np.ascontiguousarray(w1.T)          # [DIM, HID]
    w2T = np.ascontiguousarray(w2.T)          # [HID, DIM]
    gpt = np.ascontiguousarray(gamma.reshape(KT, 128).T)
    hs = HID // NCORES
    in_maps = []
    for c in range(NCORES):
        in_maps.append({
            "xT": np.ascontiguousarray(x2d[c * TOK:(c + 1) * TOK, :].T),
            "w1s": np.ascontiguousarray(w1T[:, c * hs:(c + 1) * hs]),
            "w2s": np.ascontiguousarray(w2T[c * hs:(c + 1) * hs, :]),
            "gpt": gpt,
        })
    return in_maps


def _run(in_maps, trace=False, **kw):
    nc = _get_nc()
    return bass_utils.run_bass_kernel_spmd(
        nc, in_maps, core_ids=list(range(NCORES)), trace=trace, **kw)


def kernel(x, w1, w2, gamma):
    in_maps = _prep_inputs(x, w1, w2, gamma)
    res = _run(in_maps, trace=False)
    out = np.empty((NTOK, DIM), dtype=np.float32)
    for c in range(NCORES):
        out[c * TOK:(c + 1) * TOK, :] = res.results[c]["outT"].T
    return out.reshape(B, S, DIM)



# revision 16
# speedup vs baseline: 1.1233x; 1.1233x over previous
"""BitMLP (BitNet-style MLP) Trainium2 kernel, 8-way data-parallel over tokens.

reference semantics:
  h   = act_quant(rms_norm(x, gamma)) @ w1q.T   (w1q = per-tensor ternary quant)
  out = act_quant(gelu_exact(h)) @ w2q.T

Key facts exploited:
  * act_quant produces n/scale with n an integer in [-127, 127]  -> n is exact in bf16
  * weight quant produces t*inv_w with t ternary in {-1, 0, 1}   -> t is exact in
    fp8e4, and the PE computes fp8(stationary) x bf16(moving) exactly (verified on
    HW), so ternary weights are stored/gathered/loaded in fp8 (half the bytes).
  * both matmuls are exact integer accumulations at full TensorE rate;
    per-token/per-tensor scales are applied afterwards.

Sharding (8 cores): tokens split 1024/core; weight quantization is cooperative
(each core quantizes 1/8 of w1/w2), with scalar AllReduces for the per-tensor
mean(|w|) and chunked fp8 AllGathers for the ternary weights.

Schedule notes (the previous version lost ~500us to a serialized prologue):
  * a tiny warm-up AllReduce opens the CC channel at t=0 (the first collective
    pays a ~47us barrier; overlap it with the load pass).
  * one interleaved load pass: x on the scalar DMA queue, w1+w2 on sync's; w1 is
    cached in SBUF and later ternarized IN PLACE (no second w1 read).
  * mean(|w1|) and mean(|w2|) get separate AllReduces: w1's result arrives
    ~25us earlier, and only it gates the first AllGather chunk.
  * per-engine FIFO order keeps the x->rmsnorm->act-quant path ahead of all
    AllReduce-dependent vector work, so nxT is ready before the first w1 chunk
    lands.
  * w2 quant/stores/gathers are interleaved into MM1's block loop (the CC
    stream is free then); h half-0 readback is prefetched during MM1's tail.
  * Q2 (requant of h) is produced kg-major per token-half so MM2's PSUM
    accumulation starts on the first k-blocks; half 1's production is
    interleaved into MM2 half 0's d-loop.
"""

import os
import sys

for _p in ("/root/.axon_site/_ro/trn_rl_repo", "/opt/trn_rl_repo"):
    if os.path.isdir(_p) and _p not in sys.path:
        sys.path.append(_p)

from contextlib import ExitStack

import numpy as np

from concourse import bacc, bass, masks, mybir, tile
from concourse import bass_utils

F32 = mybir.dt.float32
BF16 = mybir.dt.bfloat16
FP8 = mybir.dt.float8e4
AF = mybir.ActivationFunctionType
OP = mybir.AluOpType
AX = mybir.AxisListType

NCORES = 8
B, S, DIM, HID = 4, 2048, 2048, 8192
NTOK = B * S            # 8192
TOK = NTOK // NCORES    # 1024 tokens per core
KT = DIM // 128         # 16 k-tiles
HB = HID // 128         # 64 hid blocks
DB = DIM // 128         # 16 dim blocks
HBL = HID // NCORES // 128  # 8 hid blocks owned per core
NAG = 4                 # w1 AllGather chunks
HBC = HBL // NAG        # hid blocks per chunk per core (2)
CW = HBC * 128          # 256 hid cols per chunk
HS = HID // NCORES      # 1024 hid cols owned per core
MAGIC = 12582912.0      # 1.5 * 2**23: (v + MAGIC) - MAGIC == round-half-even(v)
EPS = 1e-6
W1_NELEM = float(DIM * HID)
W2_NELEM = float(DIM * HID)
WDT = FP8               # ternary weight storage dtype
NPRE = 8                # prefetched h k-blocks for Q2 half 0

_cache = {}


def _build(n_cores=NCORES):
    nc = bacc.Bacc("TRN2", target_bir_lowering=False, debug=False, num_devices=n_cores)
    xT = nc.dram_tensor("xT", [DIM, TOK], F32, kind="ExternalInput")
    w1s = nc.dram_tensor("w1s", [DIM, HS], F32, kind="ExternalInput")
    w2s = nc.dram_tensor("w2s", [HS, DIM], F32, kind="ExternalInput")
    gpt = nc.dram_tensor("gpt", [128, KT], F32, kind="ExternalInput")
    outT = nc.dram_tensor("outT", [DIM, TOK], F32, kind="ExternalOutput")
    rg = [list(range(n_cores))]

    with tile.TileContext(nc) as tc, ExitStack() as ctx:
        misc = ctx.enter_context(tc.tile_pool(name="misc", bufs=1))
        hpre_p = ctx.enter_context(tc.tile_pool(name="hpre", bufs=1))
        ps_mm = ctx.enter_context(tc.tile_pool(name="ps_mm", bufs=2, space="PSUM"))
        ps_tr = ctx.enter_context(tc.tile_pool(name="ps_tr", bufs=2, space="PSUM"))
        ps_ss = ctx.enter_context(tc.tile_pool(name="ps_ss", bufs=1, space="PSUM"))
        dram = ctx.enter_context(tc.tile_pool(name="dram", bufs=1, space="DRAM"))

        ident = misc.tile([128, 128], F32)
        masks.make_identity(nc, ident[:])
        zero_col = misc.tile([128, 1], F32)
        nc.vector.memset(zero_col[:], 0.0)
        ones_bf = misc.tile([128, 1], BF16)
        nc.vector.memset(ones_bf[:], 1.0)
        ones_f = misc.tile([128, 1], F32)
        nc.vector.memset(ones_f[:], 1.0)
        ones_row = misc.tile([1, 128], F32)
        nc.vector.memset(ones_row[:], 1.0)
        gam = misc.tile([128, KT], F32)
        nc.sync.dma_start(gam[:], gpt[:])

        def bcast_row(dst, src_row, n):
            """dst[128, 0:n] = broadcast of src_row[1, n] via PE outer product."""
            for o in range(0, n, 512):
                w = min(512, n - o)
                ps = ps_mm.tile([128, 512], F32, tag="mm0")
                nc.tensor.matmul(ps[:, 0:w], ones_row[:], src_row[:, o:o + w],
                                 start=True, stop=True)
                nc.scalar.activation(dst[:, o:o + w], ps[:, 0:w], AF.Copy, bias=0.0)

        # DRAM scratch
        warm_in = dram.tile([1, 1], F32)
        warm_out = dram.tile([1, 1], F32, addr_space="Shared")
        ar1_in = dram.tile([1, 1], F32)
        ar1_out = dram.tile([1, 1], F32, addr_space="Shared")
        ar2_in = dram.tile([1, 1], F32)
        ar2_out = dram.tile([1, 1], F32, addr_space="Shared")
        t1_store = [dram.tile([HBC, 128, KT * 128], WDT, name=f"t1s{i}") for i in range(NAG)]
        t1_g = [dram.tile([n_cores, HBC, 128, KT * 128], WDT, addr_space="Shared",
                          name=f"t1g{i}") for i in range(NAG)]
        t2_store = [dram.tile([DB, 128, 4 * 128], WDT, name=f"t2s{i}") for i in range(2)]
        t2_g = [dram.tile([n_cores, DB, 128, 4 * 128], WDT, addr_space="Shared",
                          name=f"t2g{i}") for i in range(2)]
        h_dram = dram.tile([HB, 128, TOK], F32)
        r1_d = dram.tile([8, 128], F32)
        r2_d = dram.tile([8, 128], F32)

        # long-lived broadcast planes / scalars
        s2r = misc.tile([128, TOK], F32)
        i2r = misc.tile([128, TOK], F32)
        invw = misc.tile([1, 2], F32)
        acc2 = misc.tile([128, TOK], F32)
        hpre = hpre_p.tile([128, NPRE * 512], F32)   # 2 MB

        # ====== CC warm-up: open the collective channel at t=0 =============
        wz = misc.tile([1, 1], F32)
        nc.vector.memset(wz[:], 0.0)
        nc.sync.dma_start(warm_in[:], wz[:])
        nc.gpsimd.collective_compute(
            "AllReduce", OP.add, replica_groups=rg, ins=[warm_in[:]], outs=[warm_out[:]])

        with ExitStack() as lctx:
            xq = lctx.enter_context(tc.tile_pool(name="xq", bufs=1))
            xpl = lctx.enter_context(tc.tile_pool(name="xpl", bufs=1))
            nxT = xq.tile([128, KT * TOK], BF16)     # 4 MB
            s1r = xpl.tile([128, TOK], F32)

            # ====== S1: one interleaved load+stats pass over w1, w2, x =====
            S1 = misc.tile([128, KT], F32)
            S2 = misc.tile([128, KT], F32)
            ss_ps0 = ps_ss.tile([1, 512], F32, tag="ss0")
            ss_ps1 = ps_ss.tile([1, 512], F32, tag="ss1")

            with ExitStack() as pctx:
                wio = pctx.enter_context(tc.tile_pool(name="wio", bufs=2))
                scr = pctx.enter_context(tc.tile_pool(name="scr", bufs=2))
                rows = pctx.enter_context(tc.tile_pool(name="rows", bufs=1))
                xtmp = pctx.enter_context(tc.tile_pool(name="xtmp", bufs=1))

                acc = xtmp.tile([128, TOK], F32)
                accmn = xtmp.tile([128, TOK], F32)
                nc.vector.memset(acc[:], 0.0)
                nc.vector.memset(accmn[:], 0.0)

                for kt in range(KT):
                    wst = wio.tile([128, HS], F32, tag="wst")
                    nc.sync.dma_start(wst[:], w1s[kt * 128:(kt + 1) * 128, :])
                    xt = scr.tile([128, TOK], F32, tag="xt")
                    nc.scalar.dma_start(xt[:], xT[kt * 128:(kt + 1) * 128, :])
                    # x stats: sumsq via PE, absmax of x*gamma via max/min pair
                    x2 = scr.tile([128, TOK], BF16, tag="x2")
                    nc.scalar.activation(x2[:], xt[:], AF.Square, bias=zero_col[:])
                    nc.tensor.matmul(ss_ps0[:], ones_bf[:], x2[:, 0:512],
                                     start=(kt == 0), stop=(kt == KT - 1))
                    nc.tensor.matmul(ss_ps1[:], ones_bf[:], x2[:, 512:1024],
                                     start=(kt == 0), stop=(kt == KT - 1))
                    xg = scr.tile([128, TOK], F32, tag="xg")
                    nc.vector.tensor_scalar(xg[:], xt[:], gam[:, kt:kt + 1], None, op0=OP.mult)
                    nc.vector.tensor_tensor(acc[:], acc[:], xg[:], op=OP.max)
                    nc.vector.tensor_tensor(accmn[:], accmn[:], xg[:], op=OP.min)
                    # w1 stats
                    nc.vector.tensor_reduce(S1[:, kt:kt + 1], wst[:], axis=AX.X, op=OP.add,
                                            apply_absolute_value=True)
                    # w2 stats, one half-row-block per kt
                    ht, hf = kt // 2, kt % 2
                    wst2 = wio.tile([128, DIM // 2], F32, tag="wst2")
                    nc.sync.dma_start(wst2[:], w2s[ht * 128:(ht + 1) * 128,
                                                   hf * 1024:(hf + 1) * 1024])
                    nc.vector.tensor_reduce(S2[:, kt:kt + 1], wst2[:], axis=AX.X, op=OP.add,
                                            apply_absolute_value=True)

                # ====== S2a: launch both mean(|w|) AllReduces ==============
                S1s = misc.tile([128, 1], F32)
                S2s = misc.tile([128, 1], F32)
                nc.vector.tensor_reduce(S1s[:], S1[:], axis=AX.X, op=OP.add)
                nc.vector.tensor_reduce(S2s[:], S2[:], axis=AX.X, op=OP.add)
                t1ps = ps_tr.tile([1, 1], F32, tag="tr")
                nc.tensor.matmul(t1ps[:], S1s[:], ones_f[:], start=True, stop=True)
                t1sb = misc.tile([1, 1], F32)
                nc.vector.tensor_copy(t1sb[:], t1ps[:])
                nc.sync.dma_start(ar1_in[:], t1sb[:])
                t2ps = ps_tr.tile([1, 1], F32, tag="tr")
                nc.tensor.matmul(t2ps[:], S2s[:], ones_f[:], start=True, stop=True)
                t2sb = misc.tile([1, 1], F32)
                nc.vector.tensor_copy(t2sb[:], t2ps[:])
                nc.sync.dma_start(ar2_in[:], t2sb[:])
                nc.gpsimd.collective_compute(
                    "AllReduce", OP.add, replica_groups=rg, ins=[ar1_in[:]], outs=[ar1_out[:]])
                # AR1 result readback rides the gpsimd queue (fires right after AR1)
                tot1 = misc.tile([1, 1], F32)
                nc.gpsimd.dma_start(tot1[:], ar1_out[:])

                # ====== S3: x rows (independent of the AllReduces) =========
                v_row = rows.tile([1, TOK], F32)
                nc.vector.tensor_scalar(v_row[:, 0:512], ss_ps0[:], 1.0 / DIM, EPS,
                                        op0=OP.mult, op1=OP.add)
                nc.vector.tensor_scalar(v_row[:, 512:1024], ss_ps1[:], 1.0 / DIM, EPS,
                                        op0=OP.mult, op1=OP.add)
                sq_row = rows.tile([1, TOK], F32)
                nc.scalar.activation(sq_row[:], v_row[:], AF.Sqrt, bias=zero_col[0:1, :])
                rscr = rows.tile([1, TOK], F32)
                rstd_row = rows.tile([1, TOK], F32)
                nc.vector.reciprocal_approx_accurate(rstd_row[:], sq_row[:], rscr[:])

                # absmax = max(max(xg), -min(xg))
                nc.vector.tensor_scalar(accmn[:], accmn[:], -1.0, None, op0=OP.mult)
                nc.vector.tensor_tensor(acc[:], acc[:], accmn[:], op=OP.max)
                m0 = misc.tile([128, 8], F32)
                for c in range(8):
                    pt = ps_tr.tile([128, 128], F32, tag="tr")
                    nc.tensor.transpose(pt[:], acc[:, c * 128:(c + 1) * 128], ident[:])
                    nc.vector.tensor_reduce(m0[:, c:c + 1], pt[:], axis=AX.X, op=OP.max)
                nc.sync.dma_start(r1_d[:].rearrange("c p -> p c"), m0[:])
                m0row = rows.tile([1, TOK], F32)
                nc.sync.dma_start(m0row[:], r1_d[:].rearrange("c p -> (c p)")[None, :])
                nc.vector.tensor_tensor(m0row[:], m0row[:], rstd_row[:], op=OP.mult)
                nc.vector.tensor_scalar(m0row[:], m0row[:], 1e-5, None, op0=OP.max)
                sx_row0 = rows.tile([1, TOK], F32)
                nc.vector.reciprocal_approx_accurate(sx_row0[:], m0row[:], rscr[:])
                nc.vector.tensor_scalar(sx_row0[:], sx_row0[:], 127.0, None, op0=OP.mult)
                inv_sx = rows.tile([1, TOK], F32)
                nc.vector.reciprocal_approx_accurate(inv_sx[:], sx_row0[:], rscr[:])
                rsx_row0 = rows.tile([1, TOK], F32)
                nc.vector.tensor_tensor(rsx_row0[:], rstd_row[:], sx_row0[:], op=OP.mult)
                rsx = xtmp.tile([128, TOK], F32)
                bcast_row(rsx, rsx_row0, TOK)

                # ====== S4: quantize x (needs only rsx) ====================
                for kt in range(KT):
                    xt = scr.tile([128, TOK], F32, tag="xt")
                    nc.scalar.dma_start(xt[:], xT[kt * 128:(kt + 1) * 128, :])
                    t = scr.tile([128, TOK], F32, tag="xg")
                    nc.vector.scalar_tensor_tensor(t[:], xt[:], gam[:, kt:kt + 1], rsx[:],
                                                   op0=OP.mult, op1=OP.mult)
                    nc.vector.tensor_scalar(nxT[:, kt * TOK:(kt + 1) * TOK], t[:],
                                            MAGIC, MAGIC, op0=OP.add, op1=OP.subtract)

                # ====== S2b: AR1 -> w1 scale (vector first waits here) =====
                nc.vector.tensor_scalar(invw[:, 0:1], tot1[:], 1.0 / W1_NELEM, 1e-5,
                                        op0=OP.mult, op1=OP.max)
                sw1 = misc.tile([1, 1], F32)
                nc.vector.reciprocal(sw1[:], invw[:, 0:1])
                swb1 = misc.tile([128, 1], F32)
                psb1 = ps_tr.tile([128, 1], F32, tag="tr")
                nc.tensor.matmul(psb1[:], ones_row[:], sw1[:], start=True, stop=True)
                nc.scalar.activation(swb1[:], psb1[:], AF.Copy, bias=0.0)

                s1_row0 = inv_sx
                nc.vector.tensor_scalar(s1_row0[:], inv_sx[:], invw[:, 0:1], None, op0=OP.mult)
                bcast_row(s1r, s1_row0, TOK)

                # ====== S5: re-read + ternarize own w1 shard, chunked AGs ==
                # (the re-read lands while DMA is otherwise idle and before
                # AR1 resolves, so it costs nothing on the critical path)
                for ci in range(NAG):
                    wch = wio.tile([128, KT * CW], F32, tag="wch")
                    for kt in range(KT):
                        nc.sync.dma_start(wch[:, kt * CW:(kt + 1) * CW],
                                          w1s[kt * 128:(kt + 1) * 128,
                                              ci * CW:(ci + 1) * CW])
                    nc.vector.tensor_scalar(wch[:], wch[:], swb1[:], -1.0, op0=OP.mult, op1=OP.max)
                    nc.vector.tensor_scalar(wch[:], wch[:], 1.0, MAGIC, op0=OP.min, op1=OP.add)
                    q8 = wio.tile([128, KT * CW], WDT, tag="wq8")
                    nc.vector.tensor_scalar(q8[:], wch[:], MAGIC, None, op0=OP.subtract)
                    for kt in range(KT):
                        nc.gpsimd.dma_start(
                            t1_store[ci][:, :, kt * 128:(kt + 1) * 128].rearrange(
                                "b k j -> k b j"),
                            q8[:, kt * CW:(kt + 1) * CW].rearrange("k (b j) -> k b j", b=HBC))
                    nc.gpsimd.collective_compute(
                        "AllGather", OP.bypass, replica_groups=rg,
                        ins=[t1_store[ci][:]], outs=[t1_g[ci][:]])
                    if ci == 0:
                        # AR2 rides the CC stream right after the first chunk;
                        # its readback is emitted later (MM1 loop) so it does
                        # not block the remaining chunk stores on gpsimd.
                        nc.gpsimd.collective_compute(
                            "AllReduce", OP.add, replica_groups=rg,
                            ins=[ar2_in[:]], outs=[ar2_out[:]])

            # ================= MM1: h = gelu((n_x @ t1) * s1) ==============
            # w2 quant / stores / gathers and the h half-0 prefetch are
            # interleaved into the block loop.
            with ExitStack() as xctx:
                scr2 = xctx.enter_context(tc.tile_pool(name="scr2", bufs=3))
                w1st = xctx.enter_context(tc.tile_pool(name="w1st", bufs=3))
                wio2 = xctx.enter_context(tc.tile_pool(name="wio2", bufs=2))

                nc.vector.memset(acc2[:], 0.0)
                blocks = [(ci, r, bi) for ci in range(NAG)
                          for r in range(n_cores) for bi in range(HBC)]
                swb2 = misc.tile([128, 1], F32)
                for bn, (ci, r, bi) in enumerate(blocks):
                    hb = r * HBL + ci * HBC + bi
                    wb = w1st.tile([128, KT * 128], WDT, tag="wb")
                    nc.sync.dma_start(wb[:], t1_g[ci][r, bi])
                    ps0 = ps_mm.tile([128, 512], F32, tag="mm0")
                    ps1 = ps_mm.tile([128, 512], F32, tag="mm1")
                    for kt in range(KT):
                        st, sp = (kt == 0), (kt == KT - 1)
                        nc.tensor.matmul(ps0[:], wb[:, kt * 128:(kt + 1) * 128],
                                         nxT[:, kt * TOK:kt * TOK + 512], start=st, stop=sp)
                        nc.tensor.matmul(ps1[:], wb[:, kt * 128:(kt + 1) * 128],
                                         nxT[:, kt * TOK + 512:kt * TOK + 1024], start=st, stop=sp)
                    g = scr2.tile([128, TOK], F32, tag="g")
                    for th, ps in ((0, ps0), (1, ps1)):
                        sl = slice(th * 512, th * 512 + 512)
                        hs = scr2.tile([128, 512], F32, tag="hs")
                        nc.vector.tensor_tensor(hs[:], ps[:], s1r[:, sl], op=OP.mult)
                        nc.scalar.activation(g[:, sl], hs[:], AF.Gelu, bias=zero_col[:])
                        ga = scr2.tile([128, 512], F32, tag="ga")
                        nc.scalar.activation(ga[:], g[:, sl], AF.Abs, bias=zero_col[:])
                        nc.vector.tensor_tensor(acc2[:, sl], acc2[:, sl], ga[:], op=OP.max)
                    nc.scalar.dma_start(h_dram[hb], g[:])

                    # ---- interleaved w2 pipeline ----
                    if bn == 2:
                        tot2 = misc.tile([1, 1], F32)
                        nc.gpsimd.dma_start(tot2[:], ar2_out[:])
                    if bn == 4:
                        nc.vector.tensor_scalar(invw[:, 1:2], tot2[:], 1.0 / W2_NELEM, 1e-5,
                                                op0=OP.mult, op1=OP.max)
                        sw2 = misc.tile([1, 1], F32)
                        nc.vector.reciprocal(sw2[:], invw[:, 1:2])
                        psb2 = ps_tr.tile([128, 1], F32, tag="tr")
                        nc.tensor.matmul(psb2[:], ones_row[:], sw2[:], start=True, stop=True)
                        nc.scalar.activation(swb2[:], psb2[:], AF.Copy, bias=0.0)
                    if 5 <= bn < 13:
                        ht = bn - 5
                        wt2 = wio2.tile([128, DIM], F32, tag="w2t")
                        nc.sync.dma_start(wt2[:], w2s[ht * 128:(ht + 1) * 128, :])
                        nc.vector.tensor_scalar(wt2[:], wt2[:], swb2[:], -1.0,
                                                op0=OP.mult, op1=OP.max)
                        nc.vector.tensor_scalar(wt2[:], wt2[:], 1.0, MAGIC,
                                                op0=OP.min, op1=OP.add)
                        q2 = wio2.tile([128, DIM], WDT, tag="wqq2")
                        nc.vector.tensor_scalar(q2[:], wt2[:], MAGIC, None, op0=OP.subtract)
                        cj, ho = ht // 4, ht % 4
                        nc.gpsimd.dma_start(
                            t2_store[cj][:, :, ho * 128:(ho + 1) * 128].rearrange(
                                "d k j -> k d j"),
                            q2[:].rearrange("k (d j) -> k d j", d=DB))
                        if ht == 3:
                            nc.gpsimd.collective_compute(
                                "AllGather", OP.bypass, replica_groups=rg,
                                ins=[t2_store[0][:]], outs=[t2_g[0][:]])
                        if ht == 7:
                            nc.gpsimd.collective_compute(
                                "AllGather", OP.bypass, replica_groups=rg,
                                ins=[t2_store[1][:]], outs=[t2_g[1][:]])
                    # ---- prefetch h half-0 k-blocks for Q2 ----
                    if bn >= HB - NPRE:
                        kg = bn - (HB - NPRE)
                        nc.gpsimd.dma_start(hpre[:, kg * 512:(kg + 1) * 512],
                                            h_dram[kg][:, 0:512])

        # ================= scales for h, then Q2 + MM2 =====================
        with ExitStack() as hctx:
            rows2 = hctx.enter_context(tc.tile_pool(name="rows2", bufs=1))
            hqp = hctx.enter_context(tc.tile_pool(name="hqp", bufs=1))
            hbk = hctx.enter_context(tc.tile_pool(name="hbk", bufs=3))
            w2st = hctx.enter_context(tc.tile_pool(name="w2st", bufs=2))
            hio = hctx.enter_context(tc.tile_pool(name="hio", bufs=3))

            m2 = rows2.tile([128, 8], F32)
            for c in range(8):
                pt = ps_tr.tile([128, 128], F32, tag="tr")
                nc.tensor.transpose(pt[:], acc2[:, c * 128:(c + 1) * 128], ident[:])
                nc.vector.tensor_reduce(m2[:, c:c + 1], pt[:], axis=AX.X, op=OP.max)
            nc.sync.dma_start(r2_d[:].rearrange("c p -> p c"), m2[:])
            m2row = rows2.tile([1, TOK], F32)
            nc.sync.dma_start(m2row[:], r2_d[:].rearrange("c p -> (c p)")[None, :])
            nc.vector.tensor_scalar(m2row[:], m2row[:], 1e-5, None, op0=OP.max)
            s2_row0 = rows2.tile([1, TOK], F32)
            i2_row0 = rows2.tile([1, TOK], F32)
            # i2_row0 doubles as scratch for the first reciprocal, m2row for
            # the second (both dead by their scratch use)
            nc.vector.reciprocal_approx_accurate(s2_row0[:], m2row[:], i2_row0[:])
            nc.vector.tensor_scalar(s2_row0[:], s2_row0[:], 127.0, None, op0=OP.mult)
            nc.vector.reciprocal_approx_accurate(i2_row0[:], s2_row0[:], m2row[:])
            nc.vector.tensor_scalar(i2_row0[:], i2_row0[:], invw[:, 1:2], None, op0=OP.mult)
            bcast_row(s2r, s2_row0, TOK)
            bcast_row(i2r, i2_row0, TOK)

            hq = [hqp.tile([128, HB * 512], BF16, tag=f"hq{t}", name=f"hq{t}")
                  for t in range(2)]

            def q2_block(th, kg):
                if th == 0 and kg < NPRE:
                    src = hpre[:, kg * 512:(kg + 1) * 512]
                else:
                    hb_t = hbk.tile([128, 512], F32, tag="hback")
                    nc.gpsimd.dma_start(hb_t[:], h_dram[kg][:, th * 512:(th + 1) * 512])
                    src = hb_t[:]
                t2v = hbk.tile([128, 512], F32, tag="ht2")
                nc.vector.tensor_tensor(t2v[:], src, s2r[:, th * 512:(th + 1) * 512], op=OP.mult)
                nc.vector.tensor_scalar(hq[th][:, kg * 512:(kg + 1) * 512], t2v[:],
                                        MAGIC, MAGIC, op0=OP.add, op1=OP.subtract)

            def mm2_d(th, d, interleave):
                to = th * 512
                wA = w2st.tile([128, n_cores * 512], WDT, tag="wA")
                nc.sync.dma_start(
                    wA[:].rearrange("k (r f) -> k r f", r=n_cores),
                    t2_g[0][:, d].rearrange("r k f -> k r f"))
                wB = w2st.tile([128, n_cores * 512], WDT, tag="wB")
                nc.sync.dma_start(
                    wB[:].rearrange("k (r f) -> k r f", r=n_cores),
                    t2_g[1][:, d].rearrange("r k f -> k r f"))
                ps = ps_mm.tile([128, 512], F32, tag="mm" + str(d % 2))
                for kg in range(HB):
                    r, ht = kg // HBL, kg % HBL
                    w_ = wA if ht < 4 else wB
                    ko = r * 512 + (ht % 4) * 128
                    nc.tensor.matmul(ps[:], w_[:, ko:ko + 128],
                                     hq[th][:, kg * 512:(kg + 1) * 512],
                                     start=(kg == 0), stop=(kg == HB - 1))
                for fn in interleave:
                    fn()
                ot = hio.tile([128, 512], F32, tag="ot")
                nc.vector.tensor_tensor(ot[:], ps[:], i2r[:, to:to + 512], op=OP.mult)
                nc.sync.dma_start(outT[d * 128:(d + 1) * 128, to:to + 512], ot[:])

            for kg in range(HB):
                q2_block(0, kg)
            for d in range(DB):
                il = []
                if 2 <= d < 13:
                    for j in range(6):
                        kg1 = (d - 2) * 6 + j
                        if kg1 < HB:
                            il.append((lambda k=kg1: q2_block(1, k)))
                mm2_d(0, d, il)
            for d in range(DB):
                mm2_d(1, d, [])

    nc.compile()
    return nc


def _get_nc():
    if "nc" not in _cache:
        _cache["nc"] = _build()
    return _cache["nc"]


def _prep_inputs(x, w1, w2, gamma):
    x2d = np.ascontiguousarray(np.asarray(x, dtype=np.float32).reshape(NTOK, DIM))
    w1 = np.asarray(w1, dtype=np.float32)
    w2 = np.asarray(w2, dtype=np.float32)
    gamma = np.asarray(gamma, dtype=np.float32)
    w1T = np.ascontiguousarray(w1.T)          # [DIM, HID]
    w2T = np.ascontiguousarray(w2.T)          # [HID, DIM]
    gpt = np.ascontiguousarray(gamma.reshape(KT, 128).T)
    in_maps = []
    for c in range(NCORES):
        in_maps.append({
            "xT": np.ascontiguousarray(x2d[c * TOK:(c + 1) * TOK, :].T),
            "w1s": np.ascontiguousarray(w1T[:, c * HS:(c + 1) * HS]),
            "w2s": np.ascontiguousarray(w2T[c * HS:(c + 1) * HS, :]),
            "gpt": gpt,
        })
    return in_maps


def _run(in_maps, trace=False, **kw):
    nc = _get_nc()
    return bass_utils.run_bass_kernel_spmd(
        nc, in_maps, core_ids=list(range(NCORES)), trace=trace, **kw)


def kernel(x, w1, w2, gamma):
    in_maps = _prep_inputs(x, w1, w2, gamma)
    res = _run(in_maps, trace=False)
    out = np.empty((NTOK, DIM), dtype=np.float32)
    for c in range(NCORES):
        out[c * TOK:(c + 1) * TOK, :] = res.results[c]["outT"].T
    return out.reshape(B, S, DIM)


# revision 22
# speedup vs baseline: 1.1394x; 1.0144x over previous
"""BitMLP (BitNet-style MLP) Trainium2 kernel, 8-way data-parallel over tokens.

reference semantics:
  h   = act_quant(rms_norm(x, gamma)) @ w1q.T   (w1q = per-tensor ternary quant)
  out = act_quant(gelu_exact(h)) @ w2q.T

Key facts exploited:
  * act_quant produces n/scale with n an integer in [-127, 127]  -> n is exact in bf16
  * weight quant produces t*inv_w with t ternary in {-1, 0, 1}   -> t is exact in
    fp8e4, and the PE computes fp8(stationary) x bf16(moving) exactly (verified on
    HW), so ternary weights are stored/gathered/loaded in fp8 (half the bytes).
  * both matmuls are exact integer accumulations at full TensorE rate;
    per-token/per-tensor scales are applied afterwards.

Sharding (8 cores): tokens split 1024/core; weight quantization is cooperative
(each core quantizes 1/8 of w1/w2), with scalar AllReduces for the per-tensor
mean(|w|) and chunked fp8 AllGathers for the ternary weights.

Schedule notes (the previous version lost ~500us to a serialized prologue):
  * a tiny warm-up AllReduce opens the CC channel at t=0 (the first collective
    pays a ~47us barrier; overlap it with the load pass).
  * one interleaved load pass: x on the scalar DMA queue, w1+w2 on sync's; w1 is
    cached in SBUF and later ternarized IN PLACE (no second w1 read).
  * mean(|w1|) and mean(|w2|) get separate AllReduces: w1's result arrives
    ~25us earlier, and only it gates the first AllGather chunk.
  * per-engine FIFO order keeps the x->rmsnorm->act-quant path ahead of all
    AllReduce-dependent vector work, so nxT is ready before the first w1 chunk
    lands.
  * w2 quant/stores/gathers are interleaved into MM1's block loop (the CC
    stream is free then); h half-0 readback is prefetched during MM1's tail.
  * Q2 (requant of h) is produced kg-major per token-half so MM2's PSUM
    accumulation starts on the first k-blocks; half 1's production is
    interleaved into MM2 half 0's d-loop.
"""

import os
import sys

for _p in ("/root/.axon_site/_ro/trn_rl_repo", "/opt/trn_rl_repo"):
    if os.path.isdir(_p) and _p not in sys.path:
        sys.path.append(_p)

from contextlib import ExitStack

import numpy as np

from concourse import bacc, bass, masks, mybir, tile
from concourse import bass_utils

F32 = mybir.dt.float32
BF16 = mybir.dt.bfloat16
FP8 = mybir.dt.float8e4
AF = mybir.ActivationFunctionType
OP = mybir.AluOpType
AX = mybir.AxisListType

NCORES = 8
B, S, DIM, HID = 4, 2048, 2048, 8192
NTOK = B * S            # 8192
TOK = NTOK // NCORES    # 1024 tokens per core
KT = DIM // 128         # 16 k-tiles
HB = HID // 128         # 64 hid blocks
DB = DIM // 128         # 16 dim blocks
HBL = HID // NCORES // 128  # 8 hid blocks owned per core
NAG = 8                 # w1 AllGather chunks
HBC = HBL // NAG        # hid blocks per chunk per core (1)
CW = HBC * 128          # 128 hid cols per chunk
HS = HID // NCORES      # 1024 hid cols owned per core
MAGIC = 12582912.0      # 1.5 * 2**23: (v + MAGIC) - MAGIC == round-half-even(v)
EPS = 1e-6
W1_NELEM = float(DIM * HID)
W2_NELEM = float(DIM * HID)
WDT = FP8               # ternary weight storage dtype
NPRE = 8                # prefetched h k-blocks for Q2 half 0

_cache = {}


def _build(n_cores=NCORES):
    nc = bacc.Bacc("TRN2", target_bir_lowering=False, debug=False, num_devices=n_cores)
    xT = nc.dram_tensor("xT", [DIM, TOK], F32, kind="ExternalInput")
    w1s = nc.dram_tensor("w1s", [DIM, HS], F32, kind="ExternalInput")
    w2s = nc.dram_tensor("w2s", [HS, DIM], F32, kind="ExternalInput")
    gpt = nc.dram_tensor("gpt", [128, KT], F32, kind="ExternalInput")
    outT = nc.dram_tensor("outT", [DIM, TOK], F32, kind="ExternalOutput")
    rg = [list(range(n_cores))]

    with tile.TileContext(nc) as tc, ExitStack() as ctx:
        misc = ctx.enter_context(tc.tile_pool(name="misc", bufs=1))
        hpre_p = ctx.enter_context(tc.tile_pool(name="hpre", bufs=1))
        ps_mm = ctx.enter_context(tc.tile_pool(name="ps_mm", bufs=2, space="PSUM"))
        ps_tr = ctx.enter_context(tc.tile_pool(name="ps_tr", bufs=2, space="PSUM"))
        ps_ss = ctx.enter_context(tc.tile_pool(name="ps_ss", bufs=1, space="PSUM"))
        dram = ctx.enter_context(tc.tile_pool(name="dram", bufs=1, space="DRAM"))

        ident = misc.tile([128, 128], F32)
        masks.make_identity(nc, ident[:])
        zero_col = misc.tile([128, 1], F32)
        nc.vector.memset(zero_col[:], 0.0)
        ones_bf = misc.tile([128, 1], BF16)
        nc.vector.memset(ones_bf[:], 1.0)
        ones_f = misc.tile([128, 1], F32)
        nc.vector.memset(ones_f[:], 1.0)
        ones_row = misc.tile([1, 128], F32)
        nc.vector.memset(ones_row[:], 1.0)
        gam = misc.tile([128, KT], F32)
        nc.sync.dma_start(gam[:], gpt[:])

        def bcast_row(dst, src_row, n):
            """dst[128, 0:n] = broadcast of src_row[1, n] via PE outer product."""
            for o in range(0, n, 512):
                w = min(512, n - o)
                ps = ps_mm.tile([128, 512], F32, tag="mm0")
                nc.tensor.matmul(ps[:, 0:w], ones_row[:], src_row[:, o:o + w],
                                 start=True, stop=True)
                nc.scalar.activation(dst[:, o:o + w], ps[:, 0:w], AF.Copy, bias=0.0)

        # DRAM scratch
        ar1_in = dram.tile([1, 1], F32)
        ar1_out = dram.tile([1, 1], F32, addr_space="Shared")
        ar2_in = dram.tile([1, 1], F32)
        ar2_out = dram.tile([1, 1], F32, addr_space="Shared")
        # t1 chunk layout is the quantizer's natural [dim-part, (kt, hid)] --
        # contiguous stores/gathers/loads, and the per-kt slice is directly
        # the matmul lhsT (no transposing DMA anywhere).
        t1_store = [dram.tile([128, KT * CW], WDT, name=f"t1s{i}") for i in range(NAG)]
        t1_g = [dram.tile([n_cores, 128, KT * CW], WDT, addr_space="Shared",
                          name=f"t1g{i}") for i in range(NAG)]
        t2_store = [dram.tile([DB, 128, 4 * 128], WDT, name=f"t2s{i}") for i in range(2)]
        t2_g = [dram.tile([n_cores, DB, 128, 4 * 128], WDT, addr_space="Shared",
                          name=f"t2g{i}") for i in range(2)]
        h_dram = dram.tile([HB, 128, TOK], F32)
        r1_d = dram.tile([8, 128], F32)
        r2_d = dram.tile([8, 128], F32)

        # long-lived broadcast planes / scalars
        s2r = misc.tile([128, TOK], F32)
        i2r = misc.tile([128, TOK], F32)
        invw = misc.tile([1, 2], F32)
        acc2 = misc.tile([128, TOK], F32)
        hpre = hpre_p.tile([128, NPRE * 512], F32)   # 2 MB

        with ExitStack() as lctx:
            xq = lctx.enter_context(tc.tile_pool(name="xq", bufs=1))
            xpl = lctx.enter_context(tc.tile_pool(name="xpl", bufs=1))
            nxT = xq.tile([128, KT * TOK], BF16)     # 4 MB
            s1r = xpl.tile([128, TOK], F32)

            # ====== S1: one interleaved load+stats pass over w1, w2, x =====
            S1 = misc.tile([128, KT], F32)
            S2 = misc.tile([128, KT], F32)
            ss_ps0 = ps_ss.tile([1, 512], F32, tag="ss0")
            ss_ps1 = ps_ss.tile([1, 512], F32, tag="ss1")

            with ExitStack() as pctx:
                wio = pctx.enter_context(tc.tile_pool(name="wio", bufs=2))
                scr = pctx.enter_context(tc.tile_pool(name="scr", bufs=2))
                rows = pctx.enter_context(tc.tile_pool(name="rows", bufs=1))
                xtmp = pctx.enter_context(tc.tile_pool(name="xtmp", bufs=1))

                acc = xtmp.tile([128, TOK], F32)
                accmn = xtmp.tile([128, TOK], F32)
                nc.vector.memset(acc[:], 0.0)
                nc.vector.memset(accmn[:], 0.0)

                # ---- w1 stats first: they alone gate AR1 -> quant -> AG ----
                for kt in range(KT):
                    wst = wio.tile([128, HS], F32, tag="wst")
                    nc.sync.dma_start(wst[:], w1s[kt * 128:(kt + 1) * 128, :])
                    nc.vector.tensor_reduce(S1[:, kt:kt + 1], wst[:], axis=AX.X, op=OP.add,
                                            apply_absolute_value=True)
                S1s = misc.tile([128, 1], F32)
                nc.vector.tensor_reduce(S1s[:], S1[:], axis=AX.X, op=OP.add)
                t1ps = ps_tr.tile([1, 1], F32, tag="tr")
                nc.tensor.matmul(t1ps[:], S1s[:], ones_f[:], start=True, stop=True)
                t1sb = misc.tile([1, 1], F32)
                nc.vector.tensor_copy(t1sb[:], t1ps[:])
                nc.sync.dma_start(ar1_in[:], t1sb[:])
                nc.gpsimd.collective_compute(
                    "AllReduce", OP.add, replica_groups=rg, ins=[ar1_in[:]], outs=[ar1_out[:]])
                # AR1 result readback rides the gpsimd queue (fires right after AR1)
                tot1 = misc.tile([1, 1], F32)
                nc.gpsimd.dma_start(tot1[:], ar1_out[:])

                # ---- x stats + w2 stats ----
                for kt in range(KT):
                    xt = scr.tile([128, TOK], F32, tag="xt")
                    nc.scalar.dma_start(xt[:], xT[kt * 128:(kt + 1) * 128, :])
                    # x stats: sumsq via PE, absmax of x*gamma via max/min pair
                    x2 = scr.tile([128, TOK], BF16, tag="x2")
                    nc.scalar.activation(x2[:], xt[:], AF.Square, bias=zero_col[:])
                    nc.tensor.matmul(ss_ps0[:], ones_bf[:], x2[:, 0:512],
                                     start=(kt == 0), stop=(kt == KT - 1))
                    nc.tensor.matmul(ss_ps1[:], ones_bf[:], x2[:, 512:1024],
                                     start=(kt == 0), stop=(kt == KT - 1))
                    xg = scr.tile([128, TOK], F32, tag="xg")
                    nc.vector.tensor_scalar(xg[:], xt[:], gam[:, kt:kt + 1], None, op0=OP.mult)
                    nc.vector.tensor_tensor(acc[:], acc[:], xg[:], op=OP.max)
                    nc.vector.tensor_tensor(accmn[:], accmn[:], xg[:], op=OP.min)
                    # w2 stats, one half-row-block per kt
                    ht, hf = kt // 2, kt % 2
                    wst2 = wio.tile([128, DIM // 2], F32, tag="wst2")
                    nc.sync.dma_start(wst2[:], w2s[ht * 128:(ht + 1) * 128,
                                                   hf * 1024:(hf + 1) * 1024])
                    nc.vector.tensor_reduce(S2[:, kt:kt + 1], wst2[:], axis=AX.X, op=OP.add,
                                            apply_absolute_value=True)

                S2s = misc.tile([128, 1], F32)
                nc.vector.tensor_reduce(S2s[:], S2[:], axis=AX.X, op=OP.add)
                t2ps = ps_tr.tile([1, 1], F32, tag="tr")
                nc.tensor.matmul(t2ps[:], S2s[:], ones_f[:], start=True, stop=True)
                t2sb = misc.tile([1, 1], F32)
                nc.vector.tensor_copy(t2sb[:], t2ps[:])
                nc.sync.dma_start(ar2_in[:], t2sb[:])

                # ====== S3: x rows (independent of the AllReduces) =========
                v_row = rows.tile([1, TOK], F32)
                nc.vector.tensor_scalar(v_row[:, 0:512], ss_ps0[:], 1.0 / DIM, EPS,
                                        op0=OP.mult, op1=OP.add)
                nc.vector.tensor_scalar(v_row[:, 512:1024], ss_ps1[:], 1.0 / DIM, EPS,
                                        op0=OP.mult, op1=OP.add)
                sq_row = rows.tile([1, TOK], F32)
                nc.scalar.activation(sq_row[:], v_row[:], AF.Sqrt, bias=zero_col[0:1, :])
                rscr = rows.tile([1, TOK], F32)
                rstd_row = rows.tile([1, TOK], F32)
                nc.vector.reciprocal_approx_accurate(rstd_row[:], sq_row[:], rscr[:])

                # absmax = max(max(xg), -min(xg))
                nc.vector.tensor_scalar(accmn[:], accmn[:], -1.0, None, op0=OP.mult)
                nc.vector.tensor_tensor(acc[:], acc[:], accmn[:], op=OP.max)
                m0 = misc.tile([128, 8], F32)
                for c in range(8):
                    pt = ps_tr.tile([128, 128], F32, tag="tr")
                    nc.tensor.transpose(pt[:], acc[:, c * 128:(c + 1) * 128], ident[:])
                    nc.vector.tensor_reduce(m0[:, c:c + 1], pt[:], axis=AX.X, op=OP.max)
                nc.sync.dma_start(r1_d[:].rearrange("c p -> p c"), m0[:])
                m0row = rows.tile([1, TOK], F32)
                nc.sync.dma_start(m0row[:], r1_d[:].rearrange("c p -> (c p)")[None, :])
                nc.vector.tensor_tensor(m0row[:], m0row[:], rstd_row[:], op=OP.mult)
                nc.vector.tensor_scalar(m0row[:], m0row[:], 1e-5, None, op0=OP.max)
                sx_row0 = rows.tile([1, TOK], F32)
                nc.vector.reciprocal_approx_accurate(sx_row0[:], m0row[:], rscr[:])
                nc.vector.tensor_scalar(sx_row0[:], sx_row0[:], 127.0, None, op0=OP.mult)
                inv_sx = rows.tile([1, TOK], F32)
                nc.vector.reciprocal_approx_accurate(inv_sx[:], sx_row0[:], rscr[:])
                rsx_row0 = rows.tile([1, TOK], F32)
                nc.vector.tensor_tensor(rsx_row0[:], rstd_row[:], sx_row0[:], op=OP.mult)
                rsx = xtmp.tile([128, TOK], F32)
                bcast_row(rsx, rsx_row0, TOK)

                # ====== S4: quantize x (needs only rsx) ====================
                for kt in range(KT):
                    xt = scr.tile([128, TOK], F32, tag="xt")
                    nc.scalar.dma_start(xt[:], xT[kt * 128:(kt + 1) * 128, :])
                    t = scr.tile([128, TOK], F32, tag="xg")
                    nc.vector.scalar_tensor_tensor(t[:], xt[:], gam[:, kt:kt + 1], rsx[:],
                                                   op0=OP.mult, op1=OP.mult)
                    nc.vector.tensor_scalar(nxT[:, kt * TOK:(kt + 1) * TOK], t[:],
                                            MAGIC, MAGIC, op0=OP.add, op1=OP.subtract)

                # ====== S2b: AR1 -> w1 scale (vector first waits here) =====
                nc.vector.tensor_scalar(invw[:, 0:1], tot1[:], 1.0 / W1_NELEM, 1e-5,
                                        op0=OP.mult, op1=OP.max)
                sw1 = misc.tile([1, 1], F32)
                nc.vector.reciprocal(sw1[:], invw[:, 0:1])
                swb1 = misc.tile([128, 1], F32)
                psb1 = ps_tr.tile([128, 1], F32, tag="tr")
                nc.tensor.matmul(psb1[:], ones_row[:], sw1[:], start=True, stop=True)
                nc.scalar.activation(swb1[:], psb1[:], AF.Copy, bias=0.0)

                s1_row0 = inv_sx
                nc.vector.tensor_scalar(s1_row0[:], inv_sx[:], invw[:, 0:1], None, op0=OP.mult)
                bcast_row(s1r, s1_row0, TOK)

                # ====== S5: re-read + ternarize own w1 shard, chunked AGs ==
                # (the re-read lands while DMA is otherwise idle and before
                # AR1 resolves, so it costs nothing on the critical path)
                for ci in range(NAG):
                    wch = wio.tile([128, KT * CW], F32, tag="wch")
                    for kt in range(KT):
                        nc.sync.dma_start(wch[:, kt * CW:(kt + 1) * CW],
                                          w1s[kt * 128:(kt + 1) * 128,
                                              ci * CW:(ci + 1) * CW])
                    nc.vector.tensor_scalar(wch[:], wch[:], swb1[:], -1.0, op0=OP.mult, op1=OP.max)
                    nc.vector.tensor_scalar(wch[:], wch[:], 1.0, MAGIC, op0=OP.min, op1=OP.add)
                    q8 = wio.tile([128, KT * CW], WDT, tag="wq8")
                    nc.vector.tensor_scalar(q8[:], wch[:], MAGIC, None, op0=OP.subtract)
                    nc.gpsimd.dma_start(t1_store[ci][:], q8[:])
                    nc.gpsimd.collective_compute(
                        "AllGather", OP.bypass, replica_groups=rg,
                        ins=[t1_store[ci][:]], outs=[t1_g[ci][:]])
                    if ci == 0:
                        # AR2 rides the CC stream right after the first chunk;
                        # its readback is emitted later (MM1 loop) so it does
                        # not block the remaining chunk stores on gpsimd.
                        nc.gpsimd.collective_compute(
                            "AllReduce", OP.add, replica_groups=rg,
                            ins=[ar2_in[:]], outs=[ar2_out[:]])

            # ================= MM1: h = gelu((n_x @ t1) * s1) ==============
            # w2 quant / stores / gathers and the h half-0 prefetch are
            # interleaved into the block loop.
            with ExitStack() as xctx:
                scr2 = xctx.enter_context(tc.tile_pool(name="scr2", bufs=3))
                w1st = xctx.enter_context(tc.tile_pool(name="w1st", bufs=3))
                wio2 = xctx.enter_context(tc.tile_pool(name="wio2", bufs=2))

                nc.vector.memset(acc2[:], 0.0)
                blocks = [(ci, r) for ci in range(NAG) for r in range(n_cores)]
                swb2 = misc.tile([128, 1], F32)
                for bn, (ci, r) in enumerate(blocks):
                    hb = r * HBL + ci
                    wb = w1st.tile([128, KT * CW], WDT, tag="wb")
                    nc.sync.dma_start(wb[:], t1_g[ci][r])
                    ps0 = ps_mm.tile([128, 512], F32, tag="mm0")
                    ps1 = ps_mm.tile([128, 512], F32, tag="mm1")
                    for kt in range(KT):
                        st, sp = (kt == 0), (kt == KT - 1)
                        nc.tensor.matmul(ps0[:], wb[:, kt * CW:(kt + 1) * CW],
                                         nxT[:, kt * TOK:kt * TOK + 512], start=st, stop=sp)
                        nc.tensor.matmul(ps1[:], wb[:, kt * CW:(kt + 1) * CW],
                                         nxT[:, kt * TOK + 512:kt * TOK + 1024], start=st, stop=sp)
                    g = scr2.tile([128, TOK], F32, tag="g")
                    for th, ps in ((0, ps0), (1, ps1)):
                        sl = slice(th * 512, th * 512 + 512)
                        hs = scr2.tile([128, 512], F32, tag="hs")
                        nc.vector.tensor_tensor(hs[:], ps[:], s1r[:, sl], op=OP.mult)
                        nc.scalar.activation(g[:, sl], hs[:], AF.Gelu, bias=zero_col[:])
                        ga = scr2.tile([128, 512], F32, tag="ga")
                        nc.scalar.activation(ga[:], g[:, sl], AF.Abs, bias=zero_col[:])
                        nc.vector.tensor_tensor(acc2[:, sl], acc2[:, sl], ga[:], op=OP.max)
                    nc.scalar.dma_start(h_dram[hb], g[:])

                    # ---- interleaved w2 pipeline ----
                    if bn == 2:
                        tot2 = misc.tile([1, 1], F32)
                        nc.gpsimd.dma_start(tot2[:], ar2_out[:])
                    if bn == 4:
                        nc.vector.tensor_scalar(invw[:, 1:2], tot2[:], 1.0 / W2_NELEM, 1e-5,
                                                op0=OP.mult, op1=OP.max)
                        sw2 = misc.tile([1, 1], F32)
                        nc.vector.reciprocal(sw2[:], invw[:, 1:2])
                        psb2 = ps_tr.tile([128, 1], F32, tag="tr")
                        nc.tensor.matmul(psb2[:], ones_row[:], sw2[:], start=True, stop=True)
                        nc.scalar.activation(swb2[:], psb2[:], AF.Copy, bias=0.0)
                    if 5 <= bn < 13:
                        ht = bn - 5
                        wt2 = wio2.tile([128, DIM], F32, tag="w2t")
                        nc.sync.dma_start(wt2[:], w2s[ht * 128:(ht + 1) * 128, :])
                        nc.vector.tensor_scalar(wt2[:], wt2[:], swb2[:], -1.0,
                                                op0=OP.mult, op1=OP.max)
                        nc.vector.tensor_scalar(wt2[:], wt2[:], 1.0, MAGIC,
                                                op0=OP.min, op1=OP.add)
                        q2 = wio2.tile([128, DIM], WDT, tag="wqq2")
                        nc.vector.tensor_scalar(q2[:], wt2[:], MAGIC, None, op0=OP.subtract)
                        cj, ho = ht // 4, ht % 4
                        nc.gpsimd.dma_start(
                            t2_store[cj][:, :, ho * 128:(ho + 1) * 128].rearrange(
                                "d k j -> k d j"),
                            q2[:].rearrange("k (d j) -> k d j", d=DB))
                        if ht == 3:
                            nc.gpsimd.collective_compute(
                                "AllGather", OP.bypass, replica_groups=rg,
                                ins=[t2_store[0][:]], outs=[t2_g[0][:]])
                        if ht == 7:
                            nc.gpsimd.collective_compute(
                                "AllGather", OP.bypass, replica_groups=rg,
                                ins=[t2_store[1][:]], outs=[t2_g[1][:]])
                    # ---- prefetch h half-0 k-blocks for Q2 ----
                    if bn >= HB - NPRE:
                        kg = bn - (HB - NPRE)
                        nc.gpsimd.dma_start(hpre[:, kg * 512:(kg + 1) * 512],
                                            h_dram[kg][:, 0:512])

        # ================= scales for h, then Q2 + MM2 =====================
        with ExitStack() as hctx:
            rows2 = hctx.enter_context(tc.tile_pool(name="rows2", bufs=1))
            hqp = hctx.enter_context(tc.tile_pool(name="hqp", bufs=1))
            hbk = hctx.enter_context(tc.tile_pool(name="hbk", bufs=3))
            w2st = hctx.enter_context(tc.tile_pool(name="w2st", bufs=2))
            hio = hctx.enter_context(tc.tile_pool(name="hio", bufs=3))

            m2 = rows2.tile([128, 8], F32)
            for c in range(8):
                pt = ps_tr.tile([128, 128], F32, tag="tr")
                nc.tensor.transpose(pt[:], acc2[:, c * 128:(c + 1) * 128], ident[:])
                nc.vector.tensor_reduce(m2[:, c:c + 1], pt[:], axis=AX.X, op=OP.max)
            nc.sync.dma_start(r2_d[:].rearrange("c p -> p c"), m2[:])
            m2row = rows2.tile([1, TOK], F32)
            nc.sync.dma_start(m2row[:], r2_d[:].rearrange("c p -> (c p)")[None, :])
            nc.vector.tensor_scalar(m2row[:], m2row[:], 1e-5, None, op0=OP.max)
            s2_row0 = rows2.tile([1, TOK], F32)
            i2_row0 = rows2.tile([1, TOK], F32)
            # i2_row0 doubles as scratch for the first reciprocal, m2row for
            # the second (both dead by their scratch use)
            nc.vector.reciprocal_approx_accurate(s2_row0[:], m2row[:], i2_row0[:])
            nc.vector.tensor_scalar(s2_row0[:], s2_row0[:], 127.0, None, op0=OP.mult)
            nc.vector.reciprocal_approx_accurate(i2_row0[:], s2_row0[:], m2row[:])
            nc.vector.tensor_scalar(i2_row0[:], i2_row0[:], invw[:, 1:2], None, op0=OP.mult)
            bcast_row(s2r, s2_row0, TOK)
            bcast_row(i2r, i2_row0, TOK)

            hq = [hqp.tile([128, HB * 512], BF16, tag=f"hq{t}", name=f"hq{t}")
                  for t in range(2)]

            def q2_block(th, kg):
                if th == 0 and kg < NPRE:
                    src = hpre[:, kg * 512:(kg + 1) * 512]
                else:
                    hb_t = hbk.tile([128, 512], F32, tag="hback")
                    nc.gpsimd.dma_start(hb_t[:], h_dram[kg][:, th * 512:(th + 1) * 512])
                    src = hb_t[:]
                t2v = hbk.tile([128, 512], F32, tag="ht2")
                nc.vector.tensor_tensor(t2v[:], src, s2r[:, th * 512:(th + 1) * 512], op=OP.mult)
                nc.vector.tensor_scalar(hq[th][:, kg * 512:(kg + 1) * 512], t2v[:],
                                        MAGIC, MAGIC, op0=OP.add, op1=OP.subtract)

            def mm2_d(th, d, interleave):
                to = th * 512
                wA = w2st.tile([128, n_cores * 512], WDT, tag="wA")
                nc.sync.dma_start(
                    wA[:].rearrange("k (r f) -> k r f", r=n_cores),
                    t2_g[0][:, d].rearrange("r k f -> k r f"))
                wB = w2st.tile([128, n_cores * 512], WDT, tag="wB")
                nc.sync.dma_start(
                    wB[:].rearrange("k (r f) -> k r f", r=n_cores),
                    t2_g[1][:, d].rearrange("r k f -> k r f"))
                ps = ps_mm.tile([128, 512], F32, tag="mm" + str(d % 2))
                for kg in range(HB):
                    r, ht = kg // HBL, kg % HBL
                    w_ = wA if ht < 4 else wB
                    ko = r * 512 + (ht % 4) * 128
                    nc.tensor.matmul(ps[:], w_[:, ko:ko + 128],
                                     hq[th][:, kg * 512:(kg + 1) * 512],
                                     start=(kg == 0), stop=(kg == HB - 1))
                for fn in interleave:
                    fn()
                ot = hio.tile([128, 512], F32, tag="ot")
                nc.vector.tensor_tensor(ot[:], ps[:], i2r[:, to:to + 512], op=OP.mult)
                nc.sync.dma_start(outT[d * 128:(d + 1) * 128, to:to + 512], ot[:])

            for kg in range(HB):
                q2_block(0, kg)
            for d in range(DB):
                il = []
                if 2 <= d < 13:
                    for j in range(6):
                        kg1 = (d - 2) * 6 + j
                        if kg1 < HB:
                            il.append((lambda k=kg1: q2_block(1, k)))
                mm2_d(0, d, il)
            for d in range(DB):
                mm2_d(1, d, [])

    nc.compile()
    return nc


def _get_nc():
    if "nc" not in _cache:
        _cache["nc"] = _build()
    return _cache["nc"]


def _prep_inputs(x, w1, w2, gamma):
    x2d = np.ascontiguousarray(np.asarray(x, dtype=np.float32).reshape(NTOK, DIM))
    w1 = np.asarray(w1, dtype=np.float32)
    w2 = np.asarray(w2, dtype=np.float32)
    gamma = np.asarray(gamma, dtype=np.float32)
    w1T = np.ascontiguousarray(w1.T)          # [DIM, HID]
    w2T = np.ascontiguousarray(w2.T)          # [HID, DIM]
    gpt = np.ascontiguousarray(gamma.reshape(KT, 128).T)
    in_maps = []
    for c in range(NCORES):
        in_maps.append({
            "xT": np.ascontiguousarray(x2d[c * TOK:(c + 1) * TOK, :].T),
            "w1s": np.ascontiguousarray(w1T[:, c * HS:(c + 1) * HS]),
            "w2s": np.ascontiguousarray(w2T[c * HS:(c + 1) * HS, :]),
            "gpt": gpt,
        })
    return in_maps


def _run(in_maps, trace=False, **kw):
    nc = _get_nc()
    return bass_utils.run_bass_kernel_spmd(
        nc, in_maps, core_ids=list(range(NCORES)), trace=trace, **kw)


def kernel(x, w1, w2, gamma):
    in_maps = _prep_inputs(x, w1, w2, gamma)
    res = _run(in_maps, trace=False)
    out = np.empty((NTOK, DIM), dtype=np.float32)
    for c in range(NCORES):
        out[c * TOK:(c + 1) * TOK, :] = res.results[c]["outT"].T
    return out.reshape(B, S, DIM)
